# revision 3
# baseline (speedup 1.0000x reference)
"""Causal self-attention (B=2, T=2048, C=768, H=12) on 8 Trainium2 cores.

Sharding: 24 (batch, head) pairs / 8 cores = 3 heads per core.
core c -> batch b = c // 4, heads [3g, 3g+3) with g = c % 4.

Per-core device program (identical SPMD program, different input data):
  qkT  = (Wqk_local^T @ x_b^T)          [384, T]   (q cols pre-scaled 1/8,
                                                    q bias added, k bias
                                                    dropped: softmax-invariant)
  V    = x_b @ Wv_local                  [T, 192]   (v bias folded on host)
  per head h:
    scoresT[k, q] = kT_h^T-block @ qT_h  (PE, K=64)
    expT = exp(scoresT)                  (ACT, causal blocks zeroed by
                                          gpsimd affine_select after exp)
    y_augT[[d;1], q] += V_aug^T @ expT   (PE, ones row -> softmax denom)
    yT_h = y_augT[0:64] * (1/denom)      (DVE; denom broadcast across
                                          partitions via a K=1 matmul)
  out_partial = Y_local @ Wp_local       [T, 768]

Host: out[b] = sum of the 4 partials + (b_proj + b_v @ W_proj).

qkT feature-chunk layout (matmul needs lhsT/rhs on the same base
partition, so each head's q and k live at the same partition offset):
  chunk0 = [q0 | q2], chunk1 = [k0 | k2], chunk2 = [q1], chunk3 = [k1]
  head 0: q (0:64, ci 0),  k (0:64, ci 1)
  head 1: q (0:64, ci 2),  k (0:64, ci 3)
  head 2: q (64:128, ci 0), k (64:128, ci 1)
"""

import numpy as np

import concourse.bass as bass
import concourse.mybir as mybir
import concourse.tile as tile
from concourse import bacc
from concourse import bass_utils

P = 128
D = 64          # head dim
HPC = 3         # heads per core
C = 768
CK = C // P     # 6 contraction chunks
QK = 2 * HPC * D  # 384 (q+k cols per core)
NH = 12
B = 2
N_CORES = 8
F32 = mybir.dt.float32

# (partition offset, chunk idx) per head, for q and k
Q_POS = [(0, 0), (0, 2), (64, 0)]
K_POS = [(0, 1), (0, 3), (64, 1)]
# wqk DRAM column ranges per chunk: (start, width)
QK_CHUNKS = [(0, 128), (128, 128), (256, 64), (320, 64)]


def build_nc(T=2048, QCW=512):
    """Build the per-core Bass program. T = sequence length, QCW = q-chunk."""
    assert T % QCW == 0 and QCW % P == 0 and T % 512 == 0
    NQC = T // QCW
    NTB = T // P
    NPH = C // 2  # 384, out-proj free-dim half

    nc = bacc.Bacc("TRN2", target_bir_lowering=False, debug=False,
                   num_devices=N_CORES)
    xT = nc.dram_tensor("xT", [C, T], F32, kind="ExternalInput").ap()
    wqk = nc.dram_tensor("wqk", [C, QK], F32, kind="ExternalInput").ap()
    wv = nc.dram_tensor("wv", [C, HPC * D], F32, kind="ExternalInput").ap()
    bqk = nc.dram_tensor("bqk", [512], F32, kind="ExternalInput").ap()
    wp = nc.dram_tensor("wp", [HPC * D, C], F32, kind="ExternalInput").ap()
    out = nc.dram_tensor("out", [T, C], F32, kind="ExternalOutput").ap()

    Exp = mybir.ActivationFunctionType.Exp

    with tile.TileContext(nc) as tc:
        with (
            tc.tile_pool(name="const", bufs=1) as const,
            tc.tile_pool(name="work", bufs=4) as work,
            tc.tile_pool(name="small", bufs=2) as small,
            tc.tile_pool(name="outp", bufs=3) as outp,
            tc.tile_pool(name="ps_mm", bufs=3, space="PSUM") as ps_mm,
            tc.tile_pool(name="ps_y", bufs=2, space="PSUM") as ps_y_pool,
            tc.tile_pool(name="ps_bc", bufs=2, space="PSUM") as ps_bc,
        ):
            xT_sb = const.tile([P, CK, T], F32, tag="xT")
            wqk_sb = const.tile([P, CK, QK], F32, tag="wqk")
            wv_sb = const.tile([P, CK, HPC * D], F32, tag="wv")
            bqk_sb = const.tile([P, 4], F32, tag="bqk")
            wp_sb = const.tile([D, HPC, C], F32, tag="wp")
            qkT_sb = const.tile([P, 4, T], F32, tag="qkT")
            v_sb = const.tile([P, NTB, HPC, D + 1], F32, tag="v")
            yT_sb = const.tile([D, HPC, T], F32, tag="yT")
            ones_sb = const.tile([1, D], F32, tag="ones")
            zb_sb = const.tile([P, 1], F32, tag="zb")

            # ---- loads ----
            for kc in range(CK):
                nc.sync.dma_start(xT_sb[:, kc, :], xT[kc * P:(kc + 1) * P, :])
            nc.sync.dma_start(wqk_sb[:], wqk.rearrange("(kc p) m -> p kc m", p=P))
            nc.sync.dma_start(wv_sb[:], wv.rearrange("(kc p) m -> p kc m", p=P))
            nc.sync.dma_start(bqk_sb[:], bqk.rearrange("(ci p) -> p ci", p=P))
            nc.sync.dma_start(wp_sb[:], wp.rearrange("(h p) e -> p h e", p=D))
            nc.gpsimd.memset(v_sb[:, :, :, D:D + 1], 1.0)
            nc.gpsimd.memset(ones_sb[:], 1.0)
            nc.gpsimd.memset(zb_sb[:], 0.0)

            # ---- qkT projection: qkT[384, T] = wqk^T @ xT (+ bias) ----
            for ci, (c0, cw) in enumerate(QK_CHUNKS):
                for tj in range(T // 512):
                    ps = ps_mm.tile([P, 512], F32, tag="mm")
                    for kc in range(CK):
                        nc.tensor.matmul(
                            ps[:cw, :],
                            wqk_sb[:, kc, c0:c0 + cw],
                            xT_sb[:, kc, tj * 512:(tj + 1) * 512],
                            start=(kc == 0), stop=(kc == CK - 1),
                        )
                    nc.vector.tensor_scalar_add(
                        qkT_sb[:cw, ci, tj * 512:(tj + 1) * 512],
                        ps[:cw, :], bqk_sb[:cw, ci:ci + 1])

            # ---- V projection: V[T, 192] = x_b @ wv ----
            for tb in range(NTB):
                ps = ps_mm.tile([P, 512], F32, tag="mm")
                for kc in range(CK):
                    nc.tensor.matmul(
                        ps[:, :HPC * D],
                        xT_sb[:, kc, tb * P:(tb + 1) * P],
                        wv_sb[:, kc, :],
                        start=(kc == 0), stop=(kc == CK - 1),
                    )
                nc.vector.tensor_copy(
                    v_sb[:, tb, :, 0:D],
                    ps[:, :HPC * D].rearrange("p (h d) -> p h d", h=HPC))

            # ---- attention + out-projection, interleaved per q-chunk ----
            for qc in range(NQC):
                q0 = qc * QCW
                kbmax = (q0 + QCW - 1) // P
                for h in range(HPC):
                    qp, qci = Q_POS[h]
                    kp, kci = K_POS[h]
                    psy = ps_y_pool.tile([P, QCW], F32, tag="yaug",
                                         name="psy")[0:D + 1, :]
                    for kb in range(kbmax + 1):
                        pss = ps_mm.tile([P, QCW], F32, tag="mm")
                        nc.tensor.matmul(
                            pss[:],
                            qkT_sb[kp:kp + D, kci, kb * P:(kb + 1) * P],
                            qkT_sb[qp:qp + D, qci, q0:q0 + QCW],
                            start=True, stop=True,
                        )
                        expT = work.tile([P, QCW], F32, tag="expT")
                        nc.scalar.activation(expT[:], pss[:], Exp,
                                             bias=zb_sb[:])
                        if kb * P >= q0:  # diagonal block: zero where q < k
                            nc.gpsimd.affine_select(
                                expT[:], expT[:],
                                pattern=[[1, QCW]],
                                compare_op=mybir.AluOpType.is_ge,
                                fill=0.0,
                                base=q0 - kb * P,
                                channel_multiplier=-1,
                            )
                        nc.tensor.matmul(
                            psy, v_sb[:, kb, h, :], expT[:],
                            start=(kb == 0), stop=(kb == kbmax),
                        )
                    # normalize: yT_h[:, q0:q0+QCW] = psy[0:D] / denom
                    recip = small.tile([1, QCW], F32, tag="recip")
                    nc.vector.reciprocal(recip[:], psy[D:D + 1, :])
                    psb = ps_bc.tile([P, QCW], F32, tag="bc",
                                     name="psb")[0:D, :]
                    nc.tensor.matmul(psb, ones_sb[:], recip[:],
                                     start=True, stop=True)
                    bc = small.tile([D, QCW], F32, tag="bcs")
                    nc.vector.tensor_copy(bc[:], psb)
                    nc.vector.tensor_mul(
                        yT_sb[:, h, q0:q0 + QCW],
                        psy[0:D, :], bc[:])

                # out-projection for the token blocks of this q-chunk
                for tb in range(q0 // P, (q0 + QCW) // P):
                    osb = outp.tile([P, C], F32, tag="osb")
                    for half in range(2):
                        pso = ps_mm.tile([P, 512], F32, tag="mm",
                                         name="pso")[:, :NPH]
                        for h in range(HPC):
                            nc.tensor.matmul(
                                pso, yT_sb[:, h, tb * P:(tb + 1) * P],
                                wp_sb[:, h, half * NPH:(half + 1) * NPH],
                                start=(h == 0), stop=(h == HPC - 1))
                        nc.vector.tensor_copy(
                            osb[:, half * NPH:(half + 1) * NPH], pso)
                    nc.sync.dma_start(out[tb * P:(tb + 1) * P, :], osb[:])

    nc.compile()
    return nc


_NC_CACHE = {}


def _get_nc(T=2048, QCW=512):
    key = (T, QCW)
    if key not in _NC_CACHE:
        _NC_CACHE[key] = build_nc(T, QCW)
    return _NC_CACHE[key]


def build_in_maps(inputs):
    """Build the 8 per-core input dicts from full inputs."""
    x = np.asarray(inputs["x"], np.float32)
    W = np.asarray(inputs["W_attn"], np.float32)
    b = np.asarray(inputs["b_attn"], np.float32)
    W_proj = np.asarray(inputs["W_proj"], np.float32)
    in_maps = []
    for c in range(N_CORES):
        bi, g = divmod(c, 4)
        lo = g * (HPC * D)  # local head col offset within each of q/k/v
        qw = [W[:, lo + i * D:lo + (i + 1) * D] * 0.125 for i in range(HPC)]
        kw = [W[:, C + lo + i * D:C + lo + (i + 1) * D] for i in range(HPC)]
        qb = [b[lo + i * D:lo + (i + 1) * D] * 0.125 for i in range(HPC)]
        # chunk order: [q0|q2], [k0|k2], [q1], [k1]
        wqk = np.concatenate([qw[0], qw[2], kw[0], kw[2], qw[1], kw[1]],
                             axis=1)
        z64 = np.zeros(D, np.float32)
        bqk = np.concatenate([qb[0], qb[2], z64, z64, qb[1], z64, z64, z64])
        wv = W[:, 2 * C + lo:2 * C + lo + HPC * D]
        wp = W_proj[lo:lo + HPC * D]
        in_maps.append({
            "xT": np.ascontiguousarray(x[bi].T),
            "wqk": np.ascontiguousarray(wqk),
            "wv": np.ascontiguousarray(wv),
            "bqk": np.ascontiguousarray(bqk),
            "wp": np.ascontiguousarray(wp),
        })
    return in_maps


def postprocess(results, inputs):
    b_attn = np.asarray(inputs["b_attn"], np.float32)
    W_proj = np.asarray(inputs["W_proj"], np.float32)
    b_proj = np.asarray(inputs["b_proj"], np.float32)
    b_eff = b_proj + b_attn[2 * C:] @ W_proj
    T = results[0]["out"].shape[0]
    out = np.zeros((B, T, C), np.float32)
    for c in range(N_CORES):
        out[c // 4] += results[c]["out"]
    out += b_eff
    return out


def kernel(x, W_attn, b_attn, W_proj, b_proj):
    inputs = dict(x=x, W_attn=W_attn, b_attn=b_attn,
                  W_proj=W_proj, b_proj=b_proj)
    T = np.asarray(x).shape[1]
    nc = _get_nc(T=T)
    in_maps = build_in_maps(inputs)
    res = bass_utils.run_bass_kernel_spmd(
        nc, in_maps, core_ids=list(range(N_CORES)))
    return postprocess(res.results, inputs)


# revision 7
# speedup vs baseline: 1.8654x; 1.8654x over previous
"""Causal self-attention (B=2, T=2048, C=768, H=12) on 8 Trainium2 cores.

Sharding: 24 (batch, head) pairs / 8 cores = 3 heads per core.
core c -> batch b = c // 4, heads [3g, 3g+3) with g = c % 4.

Per-core device program (identical SPMD program, different input data):
  qkT  = (Wqk_local^T @ x_b^T)          [384, T]   (q cols pre-scaled 1/8,
                                                    q bias added, k bias
                                                    dropped: softmax-invariant)
  V    = x_b @ Wv_local                  [T, 192]   (v bias folded on host)
  per head h:
    scoresT[k, q] = kT_h^T-block @ qT_h  (PE, K=64)
    expT = exp(scoresT)                  (ACT; causal diagonal blocks
                                          multiplied by precomputed 0/1
                                          masks on DVE)
    y_augT[[d;1], q] += V_aug^T @ expT   (PE, ones row -> softmax denom)
    yT_h = y_augT[y rows] * (1/denom)    (DVE; denom broadcast across
                                          partitions via a K=1 matmul)
  out_partial = Y_local @ Wp_local       [T, 768]

Host: out[b] = sum of the 4 partials + (b_proj + b_v @ W_proj).

Matmuls run in float32r (single-pass fp32, ~13 mantissa bits, ~2.2x
faster than the two-pass LOW_HIGH fp32 mode). Set MM_DT to
mybir.dt.float32 to go back to exact fp32.

qkT feature-chunk layout (matmul needs lhsT/rhs on the same base
partition, so each head's q and k live at the same partition offset):
  chunk0 = [q0 | q2], chunk1 = [k0 | k2], chunk2 = [q1], chunk3 = [k1]
yT layout [128, 2, T]: h0 -> (0:64, 0), h1 -> (64:128, 0), h2 -> (0:64, 1)
so the out-projection fuses h0+h1 into one K=128 matmul.
V_aug per-kb free layout [65 | 128 | 65]:
  h0: [v_h0, 1]; h1: [1, 0*63, v_h1] (y rows 64:128, denom row 0);
  h2: [v_h2, 1]
"""

import numpy as np

import concourse.bass as bass
import concourse.mybir as mybir
import concourse.tile as tile
from concourse import bacc
from concourse import bass_utils

P = 128
D = 64          # head dim
HPC = 3         # heads per core
C = 768
CK = C // P     # 6 contraction chunks
QK = 2 * HPC * D  # 384 (q+k cols per core)
NH = 12
B = 2
N_CORES = 8
F32 = mybir.dt.float32
MM_DT = mybir.dt.float32r

# (partition offset, chunk idx) per head, for q and k
Q_POS = [(0, 0), (0, 2), (64, 0)]
K_POS = [(0, 1), (0, 3), (64, 1)]
# wqk DRAM column ranges per chunk: (start, width)
QK_CHUNKS = [(0, 128), (128, 128), (256, 64), (320, 64)]
# V_aug free-layout per head: (lhsT start, lhsT width, denom row, y row0)
V_SLICE = [(0, 65, 64, 0), (65, 128, 0, 64), (193, 65, 64, 0)]
VW = 258
# yT destination (row0, chunk) per head
Y_POS = [(0, 0), (64, 0), (0, 1)]


def _r(ap):
    return ap.bitcast(MM_DT)


def build_nc(T=2048, QCW=512):
    """Build the per-core Bass program. T = sequence length, QCW = q-chunk."""
    assert T % QCW == 0 and QCW % P == 0 and T % 512 == 0
    NQC = T // QCW
    NTB = T // P
    NPH = C // 2  # 384, out-proj free-dim half

    nc = bacc.Bacc("TRN2", target_bir_lowering=False, debug=False,
                   num_devices=N_CORES)
    xT = nc.dram_tensor("xT", [C, T], F32, kind="ExternalInput").ap()
    wqk = nc.dram_tensor("wqk", [C, QK], F32, kind="ExternalInput").ap()
    wv = nc.dram_tensor("wv", [C, HPC * D], F32, kind="ExternalInput").ap()
    bqk = nc.dram_tensor("bqk", [512], F32, kind="ExternalInput").ap()
    wp = nc.dram_tensor("wp", [2 * P, C], F32, kind="ExternalInput").ap()
    out = nc.dram_tensor("out", [T, C], F32, kind="ExternalOutput").ap()

    Exp = mybir.ActivationFunctionType.Exp

    with tile.TileContext(nc) as tc:
        with (
            tc.tile_pool(name="const", bufs=1) as const,
            tc.tile_pool(name="work", bufs=4) as work,
            tc.tile_pool(name="small", bufs=2) as small,
            tc.tile_pool(name="outp", bufs=3) as outp,
            tc.tile_pool(name="ps_mm", bufs=3, space="PSUM") as ps_mm,
            tc.tile_pool(name="ps_y", bufs=2, space="PSUM") as ps_y_pool,
            tc.tile_pool(name="ps_bc", bufs=2, space="PSUM") as ps_bc,
        ):
            xT_sb = const.tile([P, CK, T], MM_DT, tag="xT")
            wqk_sb = const.tile([P, CK, QK], MM_DT, tag="wqk")
            wv_sb = const.tile([P, CK, HPC * D], MM_DT, tag="wv")
            bqk_sb = const.tile([P, 4], F32, tag="bqk")
            wp_sb = const.tile([P, 2, C], MM_DT, tag="wp")
            qkT_sb = const.tile([P, 4, T], MM_DT, tag="qkT")
            v_sb = const.tile([P, NTB, VW], MM_DT, tag="v")
            yT_sb = const.tile([P, 2, T], MM_DT, tag="yT")
            ones_sb = const.tile([1, P], MM_DT, tag="ones")
            zb_sb = const.tile([P, 1], F32, tag="zb")
            mask_sb = const.tile([P, 4, QCW], F32, tag="mask")

            # ---- loads & constants ----
            for kc in range(CK):
                nc.sync.dma_start(xT_sb[:, kc, :], xT[kc * P:(kc + 1) * P, :].bitcast(MM_DT))
            nc.sync.dma_start(wqk_sb[:], wqk.bitcast(MM_DT).rearrange("(kc p) m -> p kc m", p=P))
            nc.sync.dma_start(wv_sb[:], wv.bitcast(MM_DT).rearrange("(kc p) m -> p kc m", p=P))
            nc.sync.dma_start(bqk_sb[:], bqk.rearrange("(ci p) -> p ci", p=P))
            nc.sync.dma_start(wp_sb[:], wp.bitcast(MM_DT).rearrange("(ci p) e -> p ci e", p=P))
            # V_aug constant columns (f32r tiles can't be memset directly;
            # memset fp32 staging and DVE-copy-cast). h1 junk cols 66:129
            # are left uninitialized: they only feed psy rows 1:64, which
            # are never read.
            st = const.tile([P, 2], F32, tag="st")
            st1 = const.tile([1, P], F32, tag="st1")
            nc.gpsimd.memset(st[:], 1.0)
            nc.gpsimd.memset(st1[:], 1.0)
            nc.vector.tensor_copy(v_sb[:, :, 64:66],
                                  st[:, None, :].to_broadcast((P, NTB, 2)))
            nc.vector.tensor_copy(v_sb[:, :, 257:258],
                                  st[:, None, 0:1].to_broadcast((P, NTB, 1)))
            nc.vector.tensor_copy(ones_sb[:], st1[:])
            nc.gpsimd.memset(zb_sb[:], 0.0)
            # causal 0/1 masks for the 4 diagonal offsets:
            # mask_j[x, y] = 1 if y - x >= 128*j else 0
            nc.gpsimd.memset(mask_sb[:], 1.0)
            for j in range(4):
                nc.gpsimd.affine_select(
                    mask_sb[:, j, :], mask_sb[:, j, :],
                    pattern=[[1, QCW]],
                    compare_op=mybir.AluOpType.is_ge,
                    fill=0.0,
                    base=-128 * j,
                    channel_multiplier=-1,
                )

            # ---- qkT projection: qkT[384, T] = wqk^T @ xT (+ bias) ----
            for ci, (c0, cw) in enumerate(QK_CHUNKS):
                for tj in range(T // 512):
                    ps = ps_mm.tile([P, 512], F32, tag="mm")
                    for kc in range(CK):
                        nc.tensor.matmul(
                            ps[:cw, :],
                            wqk_sb[:, kc, c0:c0 + cw],
                            xT_sb[:, kc, tj * 512:(tj + 1) * 512],
                            start=(kc == 0), stop=(kc == CK - 1),
                        )
                    nc.vector.tensor_scalar_add(
                        qkT_sb[:cw, ci, tj * 512:(tj + 1) * 512],
                        ps[:cw, :], bqk_sb[:cw, ci:ci + 1])

            # ---- V projection: V[T, 192] = x_b @ wv ----
            for tb in range(NTB):
                ps = ps_mm.tile([P, 512], F32, tag="mm")
                for kc in range(CK):
                    nc.tensor.matmul(
                        ps[:, :HPC * D],
                        xT_sb[:, kc, tb * P:(tb + 1) * P],
                        wv_sb[:, kc, :],
                        start=(kc == 0), stop=(kc == CK - 1),
                    )
                nc.vector.tensor_copy(v_sb[:, tb, 0:64], ps[:, 0:64])
                nc.vector.tensor_copy(v_sb[:, tb, 129:257], ps[:, 64:192])

            # ---- attention + out-projection, interleaved per q-chunk ----
            for qc in range(NQC):
                q0 = qc * QCW
                kbmax = (q0 + QCW - 1) // P
                for h in range(HPC):
                    qp, qci = Q_POS[h]
                    kp, kci = K_POS[h]
                    v0, vw, srow, yrow = V_SLICE[h]
                    psy_t = ps_y_pool.tile([P, QCW], F32, tag="yaug",
                                           name="psy")
                    psy = psy_t[0:vw, :]
                    for kb in range(kbmax + 1):
                        pss = ps_mm.tile([P, QCW], F32, tag="mm")
                        nc.tensor.matmul(
                            pss[:],
                            qkT_sb[kp:kp + D, kci, kb * P:(kb + 1) * P],
                            qkT_sb[qp:qp + D, qci, q0:q0 + QCW],
                            start=True, stop=True,
                        )
                        expT = work.tile([P, QCW], MM_DT, tag="expT")
                        nc.scalar.activation(expT[:], pss[:], Exp,
                                             bias=zb_sb[:])
                        if kb * P >= q0:  # diagonal block
                            nc.vector.tensor_mul(
                                expT[:], expT[:],
                                mask_sb[:, kb - q0 // P, :])
                        nc.tensor.matmul(
                            psy, v_sb[:, kb, v0:v0 + vw], expT[:],
                            start=(kb == 0), stop=(kb == kbmax),
                        )
                    # normalize: yT_h[:, q0:q0+QCW] = y rows / denom
                    recip = small.tile([1, QCW], MM_DT, tag="recip")
                    with nc.allow_low_precision(reason="softmax denom f32r"):
                        nc.vector.reciprocal(recip[:],
                                             psy_t[srow:srow + 1, :])
                    psb = ps_bc.tile([P, QCW], F32, tag="bc", name="psb")
                    nc.tensor.matmul(psb[:], ones_sb[:], recip[:],
                                     start=True, stop=True)
                    bc = small.tile([P, QCW], F32, tag="bcs")
                    nc.vector.tensor_copy(bc[yrow:yrow + D, :],
                                          psb[yrow:yrow + D, :])
                    yp, yci = Y_POS[h]
                    nc.vector.tensor_mul(
                        yT_sb[yp:yp + D, yci, q0:q0 + QCW],
                        psy_t[yrow:yrow + D, :], bc[yrow:yrow + D, :])

                # out-projection for the token blocks of this q-chunk
                for tb in range(q0 // P, (q0 + QCW) // P):
                    osb = outp.tile([P, C], F32, tag="osb")
                    for half in range(2):
                        pso = ps_mm.tile([P, 512], F32, tag="mm",
                                         name="pso")[:, :NPH]
                        nc.tensor.matmul(
                            pso, yT_sb[:, 0, tb * P:(tb + 1) * P],
                            wp_sb[:, 0, half * NPH:(half + 1) * NPH],
                            start=True, stop=False)
                        nc.tensor.matmul(
                            pso, yT_sb[0:D, 1, tb * P:(tb + 1) * P],
                            wp_sb[0:D, 1, half * NPH:(half + 1) * NPH],
                            start=False, stop=True)
                        nc.vector.tensor_copy(
                            osb[:, half * NPH:(half + 1) * NPH], pso)
                    nc.sync.dma_start(out[tb * P:(tb + 1) * P, :], osb[:])

    nc.compile()
    return nc


_NC_CACHE = {}


def _get_nc(T=2048, QCW=512):
    key = (T, QCW)
    if key not in _NC_CACHE:
        _NC_CACHE[key] = build_nc(T, QCW)
    return _NC_CACHE[key]


def build_in_maps(inputs):
    """Build the 8 per-core input dicts from full inputs."""
    x = np.asarray(inputs["x"], np.float32)
    W = np.asarray(inputs["W_attn"], np.float32)
    b = np.asarray(inputs["b_attn"], np.float32)
    W_proj = np.asarray(inputs["W_proj"], np.float32)
    in_maps = []
    for c in range(N_CORES):
        bi, g = divmod(c, 4)
        lo = g * (HPC * D)  # local head col offset within each of q/k/v
        qw = [W[:, lo + i * D:lo + (i + 1) * D] * 0.125 for i in range(HPC)]
        kw = [W[:, C + lo + i * D:C + lo + (i + 1) * D] for i in range(HPC)]
        qb = [b[lo + i * D:lo + (i + 1) * D] * 0.125 for i in range(HPC)]
        # chunk order: [q0|q2], [k0|k2], [q1], [k1]
        wqk = np.concatenate([qw[0], qw[2], kw[0], kw[2], qw[1], kw[1]],
                             axis=1)
        z64 = np.zeros(D, np.float32)
        bqk = np.concatenate([qb[0], qb[2], z64, z64, qb[1], z64, z64, z64])
        wv = W[:, 2 * C + lo:2 * C + lo + HPC * D]
        # wp rows: [h0 | h1 | h2 | zero pad] -> chunks (0:128), (128:256)
        wp = np.zeros((2 * P, C), np.float32)
        wp[:HPC * D] = W_proj[lo:lo + HPC * D]
        in_maps.append({
            "xT": np.ascontiguousarray(x[bi].T),
            "wqk": np.ascontiguousarray(wqk),
            "wv": np.ascontiguousarray(wv),
            "bqk": np.ascontiguousarray(bqk),
            "wp": np.ascontiguousarray(wp),
        })
    return in_maps


def postprocess(results, inputs):
    b_attn = np.asarray(inputs["b_attn"], np.float32)
    W_proj = np.asarray(inputs["W_proj"], np.float32)
    b_proj = np.asarray(inputs["b_proj"], np.float32)
    b_eff = b_proj + b_attn[2 * C:] @ W_proj
    T = results[0]["out"].shape[0]
    out = np.zeros((B, T, C), np.float32)
    for c in range(N_CORES):
        out[c // 4] += results[c]["out"]
    out += b_eff
    return out


def kernel(x, W_attn, b_attn, W_proj, b_proj):
    inputs = dict(x=x, W_attn=W_attn, b_attn=b_attn,
                  W_proj=W_proj, b_proj=b_proj)
    T = np.asarray(x).shape[1]
    nc = _get_nc(T=T)
    in_maps = build_in_maps(inputs)
    res = bass_utils.run_bass_kernel_spmd(
        nc, in_maps, core_ids=list(range(N_CORES)))
    return postprocess(res.results, inputs)


# revision 9
# speedup vs baseline: 1.9550x; 1.0480x over previous
"""Causal self-attention (B=2, T=2048, C=768, H=12) on 8 Trainium2 cores.

Sharding: 24 (batch, head) pairs / 8 cores = 3 heads per core.
core c -> batch b = c // 4, heads [3g, 3g+3) with g = c % 4.

Per-core device program (identical SPMD program, different input data):
  qkT  = (Wqk_local^T @ x_b^T)          [384, T]   (q cols pre-scaled 1/8,
                                                    q bias added, k bias
                                                    dropped: softmax-invariant)
  V    = x_b @ Wv_local                  [T, 192]   (v bias folded on host)
  per head h:
    scoresT[k, q] = kT_h^T-block @ qT_h  (PE, K=64)
    expT = exp(scoresT)                  (ACT; causal diagonal blocks
                                          multiplied by precomputed 0/1
                                          masks on DVE)
    y_augT[[d;1], q] += V_aug^T @ expT   (PE, ones row -> softmax denom)
    yT_h = y_augT[y rows] * (1/denom)    (DVE; denom broadcast across
                                          partitions via a K=1 matmul)
  out_partial = Y_local @ Wp_local       [T, 768]

Host: out[b] = sum of the 4 partials + (b_proj + b_v @ W_proj).

Matmuls run in float32r (single-pass fp32, ~13 mantissa bits, ~2.2x
faster than the two-pass LOW_HIGH fp32 mode). Set MM_DT to
mybir.dt.float32 to go back to exact fp32.

qkT feature-chunk layout (matmul needs lhsT/rhs on the same base
partition, so each head's q and k live at the same partition offset):
  chunk0 = [q0 | q2], chunk1 = [k0 | k2], chunk2 = [q1], chunk3 = [k1]
yT layout [128, 2, T]: h0 -> (0:64, 0), h1 -> (64:128, 0), h2 -> (0:64, 1)
so the out-projection fuses h0+h1 into one K=128 matmul.
V_aug per-kb free layout [65 | 128 | 65]:
  h0: [v_h0, 1]; h1: [1, 0*63, v_h1] (y rows 64:128, denom row 0);
  h2: [v_h2, 1]
"""

import numpy as np

import concourse.bass as bass
import concourse.mybir as mybir
import concourse.tile as tile
from concourse import bacc
from concourse import bass_utils

P = 128
D = 64          # head dim
HPC = 3         # heads per core
C = 768
CK = C // P     # 6 contraction chunks
QK = 2 * HPC * D  # 384 (q+k cols per core)
NH = 12
B = 2
N_CORES = 8
F32 = mybir.dt.float32
MM_DT = mybir.dt.float32r

# (partition offset, chunk idx) per head, for q and k
Q_POS = [(0, 0), (0, 2), (64, 0)]
K_POS = [(0, 1), (0, 3), (64, 1)]
# wqk DRAM column ranges per chunk: (start, width)
QK_CHUNKS = [(0, 128), (128, 128), (256, 64), (320, 64)]
# V_aug free-layout per head: (lhsT start, lhsT width, denom row, y row0)
V_SLICE = [(0, 65, 64, 0), (65, 128, 0, 64), (193, 65, 64, 0)]
VW = 258
# yT destination (row0, chunk) per head
Y_POS = [(0, 0), (64, 0), (0, 1)]


def _r(ap):
    return ap.bitcast(MM_DT)


def build_nc(T=2048, QCW=512):
    """Build the per-core Bass program. T = sequence length, QCW = q-chunk."""
    assert T % QCW == 0 and QCW % P == 0 and T % 512 == 0
    NQC = T // QCW
    NTB = T // P
    NPH = C // 2  # 384, out-proj free-dim half

    nc = bacc.Bacc("TRN2", target_bir_lowering=False, debug=False,
                   num_devices=N_CORES)
    xT = nc.dram_tensor("xT", [C, T], F32, kind="ExternalInput").ap()
    wqk = nc.dram_tensor("wqk", [C, QK], F32, kind="ExternalInput").ap()
    wv = nc.dram_tensor("wv", [C, HPC * D], F32, kind="ExternalInput").ap()
    bqk = nc.dram_tensor("bqk", [512], F32, kind="ExternalInput").ap()
    wp = nc.dram_tensor("wp", [2 * P, C], F32, kind="ExternalInput").ap()
    out = nc.dram_tensor("out", [T, C], F32, kind="ExternalOutput").ap()

    Exp = mybir.ActivationFunctionType.Exp

    with tile.TileContext(nc) as tc:
        with (
            tc.tile_pool(name="const", bufs=1) as const,
            tc.tile_pool(name="work", bufs=4) as work,
            tc.tile_pool(name="small", bufs=2) as small,
            tc.tile_pool(name="outp", bufs=3) as outp,
            tc.tile_pool(name="ps_mm", bufs=4, space="PSUM") as ps_mm,
            tc.tile_pool(name="ps_y", bufs=2, space="PSUM") as ps_y_pool,
            tc.tile_pool(name="ps_bc", bufs=2, space="PSUM") as ps_bc,
        ):
            xT_sb = const.tile([P, CK, T], MM_DT, tag="xT")
            wqk_sb = const.tile([P, CK, QK], MM_DT, tag="wqk")
            wv_sb = const.tile([P, CK, HPC * D], MM_DT, tag="wv")
            bqk_sb = const.tile([P, 4], F32, tag="bqk")
            wp_sb = const.tile([P, 2, C], MM_DT, tag="wp")
            qkT_sb = const.tile([P, 4, T], MM_DT, tag="qkT")
            v_sb = const.tile([P, NTB, VW], MM_DT, tag="v")
            yT_sb = const.tile([P, 2, T], MM_DT, tag="yT")
            ones_sb = const.tile([1, P], MM_DT, tag="ones")
            zb_sb = const.tile([P, 1], F32, tag="zb")
            mask_sb = const.tile([P, 4, QCW], F32, tag="mask")

            # ---- loads & constants (weights first, xT per (kc, tj) chunk
            # so the first qkT matmuls start after ~1.5MB, not 8.5MB) ----
            nc.sync.dma_start(wqk_sb[:], wqk.bitcast(MM_DT).rearrange("(kc p) m -> p kc m", p=P))
            nc.sync.dma_start(wv_sb[:], wv.bitcast(MM_DT).rearrange("(kc p) m -> p kc m", p=P))
            nc.sync.dma_start(bqk_sb[:], bqk.rearrange("(ci p) -> p ci", p=P))
            nc.sync.dma_start(wp_sb[:], wp.bitcast(MM_DT).rearrange("(ci p) e -> p ci e", p=P))
            for tj in range(T // 512):
                for kc in range(CK):
                    nc.sync.dma_start(
                        xT_sb[:, kc, tj * 512:(tj + 1) * 512],
                        xT[kc * P:(kc + 1) * P,
                           tj * 512:(tj + 1) * 512].bitcast(MM_DT))
            # V_aug constant columns (f32r tiles can't be memset directly;
            # memset fp32 staging and DVE-copy-cast). h1 junk cols 66:129
            # are left uninitialized: they only feed psy rows 1:64, which
            # are never read.
            st = const.tile([P, 2], F32, tag="st")
            st1 = const.tile([1, P], F32, tag="st1")
            nc.gpsimd.memset(st[:], 1.0)
            nc.gpsimd.memset(st1[:], 1.0)
            nc.vector.tensor_copy(v_sb[:, :, 64:66],
                                  st[:, None, :].to_broadcast((P, NTB, 2)))
            nc.vector.tensor_copy(v_sb[:, :, 257:258],
                                  st[:, None, 0:1].to_broadcast((P, NTB, 1)))
            nc.vector.tensor_copy(ones_sb[:], st1[:])
            stz = const.tile([P, 63], F32, tag="stz")
            nc.gpsimd.memset(stz[:], 0.0)
            nc.vector.tensor_copy(v_sb[:, :, 66:129],
                                  stz[:, None, :].to_broadcast((P, NTB, 63)))
            nc.gpsimd.memset(zb_sb[:], 0.0)
            # causal 0/1 masks for the 4 diagonal offsets:
            # mask_j[x, y] = 1 if y - x >= 128*j else 0
            nc.gpsimd.memset(mask_sb[:], 1.0)
            for j in range(4):
                nc.gpsimd.affine_select(
                    mask_sb[:, j, :], mask_sb[:, j, :],
                    pattern=[[1, QCW]],
                    compare_op=mybir.AluOpType.is_ge,
                    fill=0.0,
                    base=-128 * j,
                    channel_multiplier=-1,
                )

            # ---- qkT + V projections, tj-outer so they stream with DMA ----
            for tj in range(T // 512):
                for ci, (c0, cw) in enumerate(QK_CHUNKS):
                    ps = ps_mm.tile([P, 512], F32, tag="mm")
                    for kc in range(CK):
                        nc.tensor.matmul(
                            ps[:cw, :],
                            wqk_sb[:, kc, c0:c0 + cw],
                            xT_sb[:, kc, tj * 512:(tj + 1) * 512],
                            start=(kc == 0), stop=(kc == CK - 1),
                        )
                    nc.vector.tensor_scalar_add(
                        qkT_sb[:cw, ci, tj * 512:(tj + 1) * 512],
                        ps[:cw, :], bqk_sb[:cw, ci:ci + 1])
                for tb in range(4 * tj, 4 * tj + 4):
                    ps = ps_mm.tile([P, 512], F32, tag="mm")
                    for kc in range(CK):
                        nc.tensor.matmul(
                            ps[:, :HPC * D],
                            xT_sb[:, kc, tb * P:(tb + 1) * P],
                            wv_sb[:, kc, :],
                            start=(kc == 0), stop=(kc == CK - 1),
                        )
                    nc.vector.tensor_copy(v_sb[:, tb, 0:64], ps[:, 0:64])
                    nc.vector.tensor_copy(v_sb[:, tb, 129:257],
                                          ps[:, 64:192])

            # ---- attention + out-projection, interleaved per q-chunk ----
            for qc in range(NQC):
                q0 = qc * QCW
                kbmax = (q0 + QCW - 1) // P
                for h in range(HPC):
                    qp, qci = Q_POS[h]
                    kp, kci = K_POS[h]
                    v0, vw, srow, yrow = V_SLICE[h]
                    psy_t = ps_y_pool.tile([P, QCW], F32, tag="yaug",
                                           name="psy")
                    psy = psy_t[0:vw, :]
                    for kb in range(kbmax + 1):
                        pss = ps_mm.tile([P, QCW], F32, tag="mm")
                        nc.tensor.matmul(
                            pss[:],
                            qkT_sb[kp:kp + D, kci, kb * P:(kb + 1) * P],
                            qkT_sb[qp:qp + D, qci, q0:q0 + QCW],
                            start=True, stop=True,
                        )
                        expT = work.tile([P, QCW], MM_DT, tag="expT")
                        nc.scalar.activation(expT[:], pss[:], Exp,
                                             bias=zb_sb[:])
                        if kb * P >= q0:  # diagonal block
                            nc.vector.tensor_mul(
                                expT[:], expT[:],
                                mask_sb[:, kb - q0 // P, :])
                        nc.tensor.matmul(
                            psy, v_sb[:, kb, v0:v0 + vw], expT[:],
                            start=(kb == 0), stop=(kb == kbmax),
                        )
                    # normalize: yT_h[:, q0:q0+QCW] = y rows / denom
                    recip = small.tile([1, QCW], MM_DT, tag="recip")
                    with nc.allow_low_precision(reason="softmax denom f32r"):
                        nc.vector.reciprocal(recip[:],
                                             psy_t[srow:srow + 1, :])
                    psb = ps_bc.tile([P, QCW], F32, tag="bc", name="psb")
                    nc.tensor.matmul(psb[:], ones_sb[:], recip[:],
                                     start=True, stop=True)
                    bc = small.tile([P, QCW], F32, tag="bcs")
                    nc.vector.tensor_copy(bc[yrow:yrow + D, :],
                                          psb[yrow:yrow + D, :])
                    yp, yci = Y_POS[h]
                    nc.vector.tensor_mul(
                        yT_sb[yp:yp + D, yci, q0:q0 + QCW],
                        psy_t[yrow:yrow + D, :], bc[yrow:yrow + D, :])

                # out-projection for the token blocks of this q-chunk
                for tb in range(q0 // P, (q0 + QCW) // P):
                    osb = outp.tile([P, C], F32, tag="osb")
                    for half in range(2):
                        pso = ps_mm.tile([P, 512], F32, tag="mm",
                                         name="pso")[:, :NPH]
                        nc.tensor.matmul(
                            pso, yT_sb[:, 0, tb * P:(tb + 1) * P],
                            wp_sb[:, 0, half * NPH:(half + 1) * NPH],
                            start=True, stop=False)
                        nc.tensor.matmul(
                            pso, yT_sb[0:D, 1, tb * P:(tb + 1) * P],
                            wp_sb[0:D, 1, half * NPH:(half + 1) * NPH],
                            start=False, stop=True)
                        nc.vector.tensor_copy(
                            osb[:, half * NPH:(half + 1) * NPH], pso)
                    nc.sync.dma_start(out[tb * P:(tb + 1) * P, :], osb[:])

    nc.compile()
    return nc


_NC_CACHE = {}


def _get_nc(T=2048, QCW=512):
    key = (T, QCW)
    if key not in _NC_CACHE:
        _NC_CACHE[key] = build_nc(T, QCW)
    return _NC_CACHE[key]


def build_in_maps(inputs):
    """Build the 8 per-core input dicts from full inputs."""
    x = np.asarray(inputs["x"], np.float32)
    W = np.asarray(inputs["W_attn"], np.float32)
    b = np.asarray(inputs["b_attn"], np.float32)
    W_proj = np.asarray(inputs["W_proj"], np.float32)
    in_maps = []
    for c in range(N_CORES):
        bi, g = divmod(c, 4)
        lo = g * (HPC * D)  # local head col offset within each of q/k/v
        qw = [W[:, lo + i * D:lo + (i + 1) * D] * 0.125 for i in range(HPC)]
        kw = [W[:, C + lo + i * D:C + lo + (i + 1) * D] for i in range(HPC)]
        qb = [b[lo + i * D:lo + (i + 1) * D] * 0.125 for i in range(HPC)]
        # chunk order: [q0|q2], [k0|k2], [q1], [k1]
        wqk = np.concatenate([qw[0], qw[2], kw[0], kw[2], qw[1], kw[1]],
                             axis=1)
        z64 = np.zeros(D, np.float32)
        bqk = np.concatenate([qb[0], qb[2], z64, z64, qb[1], z64, z64, z64])
        wv = W[:, 2 * C + lo:2 * C + lo + HPC * D]
        # wp rows: [h0 | h1 | h2 | zero pad] -> chunks (0:128), (128:256)
        wp = np.zeros((2 * P, C), np.float32)
        wp[:HPC * D] = W_proj[lo:lo + HPC * D]
        in_maps.append({
            "xT": np.ascontiguousarray(x[bi].T),
            "wqk": np.ascontiguousarray(wqk),
            "wv": np.ascontiguousarray(wv),
            "bqk": np.ascontiguousarray(bqk),
            "wp": np.ascontiguousarray(wp),
        })
    return in_maps


def postprocess(results, inputs):
    b_attn = np.asarray(inputs["b_attn"], np.float32)
    W_proj = np.asarray(inputs["W_proj"], np.float32)
    b_proj = np.asarray(inputs["b_proj"], np.float32)
    b_eff = b_proj + b_attn[2 * C:] @ W_proj
    T = results[0]["out"].shape[0]
    out = np.zeros((B, T, C), np.float32)
    for c in range(N_CORES):
        out[c // 4] += results[c]["out"]
    out += b_eff
    return out


def kernel(x, W_attn, b_attn, W_proj, b_proj):
    inputs = dict(x=x, W_attn=W_attn, b_attn=b_attn,
                  W_proj=W_proj, b_proj=b_proj)
    T = np.asarray(x).shape[1]
    nc = _get_nc(T=T)
    in_maps = build_in_maps(inputs)
    res = bass_utils.run_bass_kernel_spmd(
        nc, in_maps, core_ids=list(range(N_CORES)))
    return postprocess(res.results, inputs)


# revision 10
# speedup vs baseline: 1.9639x; 1.0045x over previous
"""Causal self-attention (B=2, T=2048, C=768, H=12) on 8 Trainium2 cores.

Sharding: 24 (batch, head) pairs / 8 cores = 3 heads per core.
core c -> batch b = c // 4, heads [3g, 3g+3) with g = c % 4.

Per-core device program (identical SPMD program, different input data):
  qkT  = (Wqk_local^T @ x_b^T)          [384, T]   (q cols pre-scaled 1/8,
                                                    q bias added, k bias
                                                    dropped: softmax-invariant)
  V    = x_b @ Wv_local                  [T, 192]   (v bias folded on host)
  per head h:
    scoresT[k, q] = kT_h^T-block @ qT_h  (PE, K=64)
    expT = exp(scoresT)                  (ACT; causal diagonal blocks
                                          multiplied by precomputed 0/1
                                          masks on DVE)
    y_augT[[d;1], q] += V_aug^T @ expT   (PE, ones row -> softmax denom)
    yT_h = y_augT[y rows] * (1/denom)    (DVE; denom broadcast across
                                          partitions via a K=1 matmul)
  out_partial = Y_local @ Wp_local       [T, 768]

Host: out[b] = sum of the 4 partials + (b_proj + b_v @ W_proj).

Matmuls run in float32r (single-pass fp32, ~13 mantissa bits, ~2.2x
faster than the two-pass LOW_HIGH fp32 mode). Set MM_DT to
mybir.dt.float32 to go back to exact fp32.

qkT feature-chunk layout (matmul needs lhsT/rhs on the same base
partition, so each head's q and k live at the same partition offset):
  chunk0 = [q0 | q2], chunk1 = [k0 | k2], chunk2 = [q1], chunk3 = [k1]
yT layout [128, 2, T]: h0 -> (0:64, 0), h1 -> (64:128, 0), h2 -> (0:64, 1)
so the out-projection fuses h0+h1 into one K=128 matmul.
V_aug per-kb free layout [65 | 128 | 65]:
  h0: [v_h0, 1]; h1: [1, 0*63, v_h1] (y rows 64:128, denom row 0);
  h2: [v_h2, 1]
"""

import numpy as np

import concourse.bass as bass
import concourse.mybir as mybir
import concourse.tile as tile
from concourse import bacc
from concourse import bass_utils

P = 128
D = 64          # head dim
HPC = 3         # heads per core
C = 768
CK = C // P     # 6 contraction chunks
QK = 2 * HPC * D  # 384 (q+k cols per core)
NH = 12
B = 2
N_CORES = 8
F32 = mybir.dt.float32
MM_DT = mybir.dt.float32r

# (partition offset, chunk idx) per head, for q and k
Q_POS = [(0, 0), (0, 2), (64, 0)]
K_POS = [(0, 1), (0, 3), (64, 1)]
# wqk DRAM column ranges per chunk: (start, width)
QK_CHUNKS = [(0, 128), (128, 128), (256, 64), (320, 64)]
# V_aug free-layout per head: (lhsT start, lhsT width, denom row, y row0)
V_SLICE = [(0, 65, 64, 0), (65, 128, 0, 64), (193, 65, 64, 0)]
VW = 258
# yT destination (row0, chunk) per head
Y_POS = [(0, 0), (64, 0), (0, 1)]


def _r(ap):
    return ap.bitcast(MM_DT)


def build_nc(T=2048, QCW=512):
    """Build the per-core Bass program. T = sequence length, QCW = q-chunk."""
    assert T % QCW == 0 and QCW % P == 0 and T % 512 == 0
    NQC = T // QCW
    NTB = T // P
    NPH = C // 2  # 384, out-proj free-dim half

    nc = bacc.Bacc("TRN2", target_bir_lowering=False, debug=False,
                   num_devices=N_CORES)
    xT = nc.dram_tensor("xT", [C, T], F32, kind="ExternalInput").ap()
    wqk = nc.dram_tensor("wqk", [C, QK], F32, kind="ExternalInput").ap()
    wv = nc.dram_tensor("wv", [C, HPC * D], F32, kind="ExternalInput").ap()
    bqk = nc.dram_tensor("bqk", [512], F32, kind="ExternalInput").ap()
    wp = nc.dram_tensor("wp", [2 * P, C], F32, kind="ExternalInput").ap()
    out = nc.dram_tensor("out", [T, C], F32, kind="ExternalOutput").ap()

    Exp = mybir.ActivationFunctionType.Exp

    with tile.TileContext(nc) as tc:
        with (
            tc.tile_pool(name="const", bufs=1) as const,
            tc.tile_pool(name="work", bufs=4) as work,
            tc.tile_pool(name="small", bufs=2) as small,
            tc.tile_pool(name="outp", bufs=3) as outp,
            tc.tile_pool(name="ps_mm", bufs=4, space="PSUM") as ps_mm,
            tc.tile_pool(name="ps_y", bufs=2, space="PSUM") as ps_y_pool,
            tc.tile_pool(name="ps_bc", bufs=2, space="PSUM") as ps_bc,
        ):
            xT_sb = const.tile([P, CK, T], MM_DT, tag="xT")
            wqk_sb = const.tile([P, CK, QK], MM_DT, tag="wqk")
            wv_sb = const.tile([P, CK, HPC * D], MM_DT, tag="wv")
            bqk_sb = const.tile([P, 4], F32, tag="bqk")
            wp_sb = const.tile([P, 2, C], MM_DT, tag="wp")
            qkT_sb = const.tile([P, 4, T], MM_DT, tag="qkT")
            v_sb = const.tile([P, NTB, VW], MM_DT, tag="v")
            yT_sb = const.tile([P, 2, T], MM_DT, tag="yT")
            ones_sb = const.tile([1, P], MM_DT, tag="ones")
            zb_sb = const.tile([P, 1], F32, tag="zb")
            mask_sb = const.tile([P, 4, QCW], F32, tag="mask")

            # ---- loads & constants (weights first, xT per (kc, tj) chunk
            # so the first qkT matmuls start after ~1.5MB, not 8.5MB) ----
            nc.sync.dma_start(wqk_sb[:], wqk.bitcast(MM_DT).rearrange("(kc p) m -> p kc m", p=P))
            nc.sync.dma_start(wv_sb[:], wv.bitcast(MM_DT).rearrange("(kc p) m -> p kc m", p=P))
            nc.sync.dma_start(bqk_sb[:], bqk.rearrange("(ci p) -> p ci", p=P))
            nc.sync.dma_start(wp_sb[:], wp.bitcast(MM_DT).rearrange("(ci p) e -> p ci e", p=P))
            for tj in range(T // 512):
                for kc in range(CK):
                    nc.sync.dma_start(
                        xT_sb[:, kc, tj * 512:(tj + 1) * 512],
                        xT[kc * P:(kc + 1) * P,
                           tj * 512:(tj + 1) * 512].bitcast(MM_DT))
            # V_aug constant columns (f32r tiles can't be memset directly;
            # memset fp32 staging and DVE-copy-cast). h1 junk cols 66:129
            # are left uninitialized: they only feed psy rows 1:64, which
            # are never read.
            st = const.tile([P, 2], F32, tag="st")
            st1 = const.tile([1, P], F32, tag="st1")
            nc.gpsimd.memset(st[:], 1.0)
            nc.gpsimd.memset(st1[:], 1.0)
            nc.vector.tensor_copy(v_sb[:, :, 64:66],
                                  st[:, None, :].to_broadcast((P, NTB, 2)))
            nc.vector.tensor_copy(v_sb[:, :, 257:258],
                                  st[:, None, 0:1].to_broadcast((P, NTB, 1)))
            nc.vector.tensor_copy(ones_sb[:], st1[:])
            stz = const.tile([P, 63], F32, tag="stz")
            nc.gpsimd.memset(stz[:], 0.0)
            nc.vector.tensor_copy(v_sb[:, :, 66:129],
                                  stz[:, None, :].to_broadcast((P, NTB, 63)))
            nc.gpsimd.memset(zb_sb[:], 0.0)
            # causal 0/1 masks for the 4 diagonal offsets:
            # mask_j[x, y] = 1 if y - x >= 128*j else 0
            nc.gpsimd.memset(mask_sb[:], 1.0)
            for j in range(4):
                nc.gpsimd.affine_select(
                    mask_sb[:, j, :], mask_sb[:, j, :],
                    pattern=[[1, QCW]],
                    compare_op=mybir.AluOpType.is_ge,
                    fill=0.0,
                    base=-128 * j,
                    channel_multiplier=-1,
                )

            # ---- qkT + V projections, tj-outer so they stream with DMA ----
            for tj in range(T // 512):
                for ci, (c0, cw) in enumerate(QK_CHUNKS):
                    ps = ps_mm.tile([P, 512], F32, tag="mm")
                    for kc in range(CK):
                        nc.tensor.matmul(
                            ps[:cw, :],
                            wqk_sb[:, kc, c0:c0 + cw],
                            xT_sb[:, kc, tj * 512:(tj + 1) * 512],
                            start=(kc == 0), stop=(kc == CK - 1),
                        )
                    nc.vector.tensor_scalar_add(
                        qkT_sb[:cw, ci, tj * 512:(tj + 1) * 512],
                        ps[:cw, :], bqk_sb[:cw, ci:ci + 1])
                for tb in range(4 * tj, 4 * tj + 4):
                    ps = ps_mm.tile([P, 512], F32, tag="mm")
                    for kc in range(CK):
                        nc.tensor.matmul(
                            ps[:, :HPC * D],
                            xT_sb[:, kc, tb * P:(tb + 1) * P],
                            wv_sb[:, kc, :],
                            start=(kc == 0), stop=(kc == CK - 1),
                        )
                    nc.vector.tensor_copy(v_sb[:, tb, 0:64], ps[:, 0:64])
                    nc.vector.tensor_copy(v_sb[:, tb, 129:257],
                                          ps[:, 64:192])

            # ---- attention + out-projection, interleaved per q-chunk ----
            for qc in range(NQC):
                q0 = qc * QCW
                kbmax = (q0 + QCW - 1) // P
                for h in range(HPC):
                    qp, qci = Q_POS[h]
                    kp, kci = K_POS[h]
                    v0, vw, srow, yrow = V_SLICE[h]
                    psy_t = ps_y_pool.tile([P, QCW], F32, tag="yaug",
                                           name="psy")
                    psy = psy_t[0:vw, :]
                    for kb in range(kbmax + 1):
                        pss = ps_mm.tile([P, QCW], F32, tag="mm")
                        nc.tensor.matmul(
                            pss[:],
                            qkT_sb[kp:kp + D, kci, kb * P:(kb + 1) * P],
                            qkT_sb[qp:qp + D, qci, q0:q0 + QCW],
                            start=True, stop=True,
                        )
                        expT = work.tile([P, QCW], MM_DT, tag="expT")
                        nc.scalar.activation(expT[:], pss[:], Exp,
                                             bias=zb_sb[:])
                        if kb * P >= q0:  # diagonal block
                            nc.vector.tensor_mul(
                                expT[:], expT[:],
                                mask_sb[:, kb - q0 // P, :])
                        nc.tensor.matmul(
                            psy, v_sb[:, kb, v0:v0 + vw], expT[:],
                            start=(kb == 0), stop=(kb == kbmax),
                        )
                    # normalize: yT_h[:, q0:q0+QCW] = y rows / denom
                    recip = small.tile([1, QCW], MM_DT, tag="recip")
                    with nc.allow_low_precision(reason="softmax denom f32r"):
                        nc.vector.reciprocal(recip[:],
                                             psy_t[srow:srow + 1, :])
                    psb = ps_bc.tile([P, QCW], F32, tag="bc", name="psb")
                    nc.tensor.matmul(psb[:], ones_sb[:], recip[:],
                                     start=True, stop=True)
                    bc = small.tile([P, QCW], F32, tag="bcs")
                    nc.scalar.copy(bc[yrow:yrow + D, :],
                                   psb[yrow:yrow + D, :])
                    yp, yci = Y_POS[h]
                    nc.vector.tensor_mul(
                        yT_sb[yp:yp + D, yci, q0:q0 + QCW],
                        psy_t[yrow:yrow + D, :], bc[yrow:yrow + D, :])

                # out-projection for the token blocks of this q-chunk
                for tb in range(q0 // P, (q0 + QCW) // P):
                    osb = outp.tile([P, C], F32, tag="osb")
                    for half in range(2):
                        pso = ps_mm.tile([P, 512], F32, tag="mm",
                                         name="pso")[:, :NPH]
                        nc.tensor.matmul(
                            pso, yT_sb[:, 0, tb * P:(tb + 1) * P],
                            wp_sb[:, 0, half * NPH:(half + 1) * NPH],
                            start=True, stop=False)
                        nc.tensor.matmul(
                            pso, yT_sb[0:D, 1, tb * P:(tb + 1) * P],
                            wp_sb[0:D, 1, half * NPH:(half + 1) * NPH],
                            start=False, stop=True)
                        nc.vector.tensor_copy(
                            osb[:, half * NPH:(half + 1) * NPH], pso)
                    nc.sync.dma_start(out[tb * P:(tb + 1) * P, :], osb[:])

    nc.compile()
    return nc


_NC_CACHE = {}


def _get_nc(T=2048, QCW=512):
    key = (T, QCW)
    if key not in _NC_CACHE:
        _NC_CACHE[key] = build_nc(T, QCW)
    return _NC_CACHE[key]


def build_in_maps(inputs):
    """Build the 8 per-core input dicts from full inputs."""
    x = np.asarray(inputs["x"], np.float32)
    W = np.asarray(inputs["W_attn"], np.float32)
    b = np.asarray(inputs["b_attn"], np.float32)
    W_proj = np.asarray(inputs["W_proj"], np.float32)
    in_maps = []
    for c in range(N_CORES):
        bi, g = divmod(c, 4)
        lo = g * (HPC * D)  # local head col offset within each of q/k/v
        qw = [W[:, lo + i * D:lo + (i + 1) * D] * 0.125 for i in range(HPC)]
        kw = [W[:, C + lo + i * D:C + lo + (i + 1) * D] for i in range(HPC)]
        qb = [b[lo + i * D:lo + (i + 1) * D] * 0.125 for i in range(HPC)]
        # chunk order: [q0|q2], [k0|k2], [q1], [k1]
        wqk = np.concatenate([qw[0], qw[2], kw[0], kw[2], qw[1], kw[1]],
                             axis=1)
        z64 = np.zeros(D, np.float32)
        bqk = np.concatenate([qb[0], qb[2], z64, z64, qb[1], z64, z64, z64])
        wv = W[:, 2 * C + lo:2 * C + lo + HPC * D]
        # wp rows: [h0 | h1 | h2 | zero pad] -> chunks (0:128), (128:256)
        wp = np.zeros((2 * P, C), np.float32)
        wp[:HPC * D] = W_proj[lo:lo + HPC * D]
        in_maps.append({
            "xT": np.ascontiguousarray(x[bi].T),
            "wqk": np.ascontiguousarray(wqk),
            "wv": np.ascontiguousarray(wv),
            "bqk": np.ascontiguousarray(bqk),
            "wp": np.ascontiguousarray(wp),
        })
    return in_maps


def postprocess(results, inputs):
    b_attn = np.asarray(inputs["b_attn"], np.float32)
    W_proj = np.asarray(inputs["W_proj"], np.float32)
    b_proj = np.asarray(inputs["b_proj"], np.float32)
    b_eff = b_proj + b_attn[2 * C:] @ W_proj
    T = results[0]["out"].shape[0]
    out = np.zeros((B, T, C), np.float32)
    for c in range(N_CORES):
        out[c // 4] += results[c]["out"]
    out += b_eff
    return out


def kernel(x, W_attn, b_attn, W_proj, b_proj):
    inputs = dict(x=x, W_attn=W_attn, b_attn=b_attn,
                  W_proj=W_proj, b_proj=b_proj)
    T = np.asarray(x).shape[1]
    nc = _get_nc(T=T)
    in_maps = build_in_maps(inputs)
    res = bass_utils.run_bass_kernel_spmd(
        nc, in_maps, core_ids=list(range(N_CORES)))
    return postprocess(res.results, inputs)


# revision 11
# speedup vs baseline: 2.0498x; 1.0437x over previous
"""Causal self-attention (B=2, T=2048, C=768, H=12) on 8 Trainium2 cores.

Sharding: 24 (batch, head) pairs / 8 cores = 3 heads per core.
core c -> batch b = c // 4, heads [3g, 3g+3) with g = c % 4.

Per-core device program (identical SPMD program, different input data):
  qkT  = (Wqk_local^T @ x_b^T)          [384, T]   (q cols pre-scaled 1/8,
                                                    q bias added, k bias
                                                    dropped: softmax-invariant)
  V    = x_b @ Wv_local                  [T, 192]   (v bias folded on host)
  per head h:
    scoresT[k, q] = kT_h^T-block @ qT_h  (PE, K=64)
    expT = exp(scoresT)                  (ACT; causal diagonal blocks
                                          multiplied by precomputed 0/1
                                          masks on DVE)
    y_augT[[d;1], q] += V_aug^T @ expT   (PE, ones row -> softmax denom)
    yT_h = y_augT[y rows] * (1/denom)    (DVE; denom broadcast across
                                          partitions via a K=1 matmul)
  out_partial = Y_local @ Wp_local       [T, 768]

Host: out[b] = sum of the 4 partials + (b_proj + b_v @ W_proj).

Matmuls run in float32r (single-pass fp32, ~13 mantissa bits, ~2.2x
faster than the two-pass LOW_HIGH fp32 mode). Set MM_DT to
mybir.dt.float32 to go back to exact fp32.

qkT feature-chunk layout (matmul needs lhsT/rhs on the same base
partition, so each head's q and k live at the same partition offset):
  chunk0 = [q0 | q2], chunk1 = [k0 | k2], chunk2 = [q1], chunk3 = [k1]
yT layout [128, 2, T]: h0 -> (0:64, 0), h1 -> (64:128, 0), h2 -> (0:64, 1)
so the out-projection fuses h0+h1 into one K=128 matmul.
V_aug per-kb free layout [65 | 128 | 65]:
  h0: [v_h0, 1]; h1: [1, 0*63, v_h1] (y rows 64:128, denom row 0);
  h2: [v_h2, 1]
"""

import numpy as np

import concourse.bass as bass
import concourse.mybir as mybir
import concourse.tile as tile
from concourse import bacc
from concourse import bass_utils

P = 128
D = 64          # head dim
HPC = 3         # heads per core
C = 768
CK = C // P     # 6 contraction chunks
QK = 2 * HPC * D  # 384 (q+k cols per core)
NH = 12
B = 2
N_CORES = 8
F32 = mybir.dt.float32
MM_DT = mybir.dt.float32r

# (partition offset, chunk idx) per head, for q and k
Q_POS = [(0, 0), (0, 2), (64, 0)]
K_POS = [(0, 1), (0, 3), (64, 1)]
# wqk DRAM column ranges per chunk: (start, width)
QK_CHUNKS = [(0, 128), (128, 128), (256, 64), (320, 64)]
# V_aug free-layout per head: (lhsT start, lhsT width, denom row, y row0)
V_SLICE = [(0, 65, 64, 0), (65, 128, 0, 64), (193, 65, 64, 0)]
VW = 258
# yT destination (row0, chunk) per head
Y_POS = [(0, 0), (64, 0), (0, 1)]


def _r(ap):
    return ap.bitcast(MM_DT)


def build_nc(T=2048, QCW=512):
    """Build the per-core Bass program. T = sequence length, QCW = q-chunk."""
    assert T % QCW == 0 and QCW % P == 0 and T % 512 == 0
    NQC = T // QCW
    NTB = T // P
    NPH = C // 2  # 384, out-proj free-dim half

    nc = bacc.Bacc("TRN2", target_bir_lowering=False, debug=False,
                   num_devices=N_CORES)
    xT = nc.dram_tensor("xT", [C, T], F32, kind="ExternalInput").ap()
    wqk = nc.dram_tensor("wqk", [C, QK], F32, kind="ExternalInput").ap()
    wv = nc.dram_tensor("wv", [C, HPC * D], F32, kind="ExternalInput").ap()
    bqk = nc.dram_tensor("bqk", [512], F32, kind="ExternalInput").ap()
    wp = nc.dram_tensor("wp", [2 * P, C], F32, kind="ExternalInput").ap()
    out = nc.dram_tensor("out", [T, C], F32, kind="ExternalOutput").ap()

    Exp = mybir.ActivationFunctionType.Exp

    with tile.TileContext(nc) as tc:
        with (
            tc.tile_pool(name="const", bufs=1) as const,
            tc.tile_pool(name="work", bufs=4) as work,
            tc.tile_pool(name="small", bufs=2) as small,
            tc.tile_pool(name="outp", bufs=3) as outp,
            tc.tile_pool(name="ps_mm", bufs=4, space="PSUM") as ps_mm,
            tc.tile_pool(name="ps_y", bufs=2, space="PSUM") as ps_y_pool,
            tc.tile_pool(name="ps_bc", bufs=2, space="PSUM") as ps_bc,
        ):
            xT_sb = const.tile([P, CK, T], MM_DT, tag="xT")
            wqk_sb = const.tile([P, CK, QK], MM_DT, tag="wqk")
            wv_sb = const.tile([P, CK, HPC * D], MM_DT, tag="wv")
            bqk_sb = const.tile([P, 4], F32, tag="bqk")
            wp_sb = const.tile([P, 2, C], MM_DT, tag="wp")
            qkT_sb = const.tile([P, 4, T], MM_DT, tag="qkT")
            v_sb = const.tile([P, NTB, VW], MM_DT, tag="v")
            yT_sb = const.tile([P, 2, T], MM_DT, tag="yT")
            ones_sb = const.tile([1, P], MM_DT, tag="ones")
            zb_sb = const.tile([P, 1], F32, tag="zb")
            mask_sb = const.tile([P, 4, QCW], F32, tag="mask")

            # ---- loads & constants (weights first, xT per (kc, tj) chunk
            # so the first qkT matmuls start after ~1.5MB, not 8.5MB) ----
            nc.sync.dma_start(wqk_sb[:], wqk.bitcast(MM_DT).rearrange("(kc p) m -> p kc m", p=P))
            nc.sync.dma_start(wv_sb[:], wv.bitcast(MM_DT).rearrange("(kc p) m -> p kc m", p=P))
            nc.sync.dma_start(bqk_sb[:], bqk.rearrange("(ci p) -> p ci", p=P))
            nc.sync.dma_start(wp_sb[:], wp.bitcast(MM_DT).rearrange("(ci p) e -> p ci e", p=P))
            for tj in range(T // 512):
                for kc in range(CK):
                    nc.sync.dma_start(
                        xT_sb[:, kc, tj * 512:(tj + 1) * 512],
                        xT[kc * P:(kc + 1) * P,
                           tj * 512:(tj + 1) * 512].bitcast(MM_DT))
            # V_aug constant columns (f32r tiles can't be memset directly;
            # memset fp32 staging and DVE-copy-cast). h1 junk cols 66:129
            # are left uninitialized: they only feed psy rows 1:64, which
            # are never read.
            st = const.tile([P, 2], F32, tag="st")
            st1 = const.tile([1, P], F32, tag="st1")
            nc.gpsimd.memset(st[:], 1.0)
            nc.gpsimd.memset(st1[:], 1.0)
            nc.vector.tensor_copy(v_sb[:, :, 64:66],
                                  st[:, None, :].to_broadcast((P, NTB, 2)))
            nc.vector.tensor_copy(v_sb[:, :, 257:258],
                                  st[:, None, 0:1].to_broadcast((P, NTB, 1)))
            nc.vector.tensor_copy(ones_sb[:], st1[:])
            stz = const.tile([P, 63], F32, tag="stz")
            nc.gpsimd.memset(stz[:], 0.0)
            nc.vector.tensor_copy(v_sb[:, :, 66:129],
                                  stz[:, None, :].to_broadcast((P, NTB, 63)))
            nc.gpsimd.memset(zb_sb[:], 0.0)
            # causal 0/1 masks for the 4 diagonal offsets:
            # mask_j[x, y] = 1 if y - x >= 128*j else 0
            nc.gpsimd.memset(mask_sb[:], 1.0)
            for j in range(4):
                nc.gpsimd.affine_select(
                    mask_sb[:, j, :], mask_sb[:, j, :],
                    pattern=[[1, QCW]],
                    compare_op=mybir.AluOpType.is_ge,
                    fill=0.0,
                    base=-128 * j,
                    channel_multiplier=-1,
                )

            # ---- qkT + V projections, tj-outer so they stream with DMA ----
            for tj in range(T // 512):
                for ci, (c0, cw) in enumerate(QK_CHUNKS):
                    ps = ps_mm.tile([P, 512], F32, tag="mm")
                    for kc in range(CK):
                        nc.tensor.matmul(
                            ps[:cw, :],
                            wqk_sb[:, kc, c0:c0 + cw],
                            xT_sb[:, kc, tj * 512:(tj + 1) * 512],
                            start=(kc == 0), stop=(kc == CK - 1),
                        )
                    nc.vector.tensor_scalar_add(
                        qkT_sb[:cw, ci, tj * 512:(tj + 1) * 512],
                        ps[:cw, :], bqk_sb[:cw, ci:ci + 1])
                for tb in range(4 * tj, 4 * tj + 4):
                    ps = ps_mm.tile([P, 512], F32, tag="mm")
                    for kc in range(CK):
                        nc.tensor.matmul(
                            ps[:, :HPC * D],
                            xT_sb[:, kc, tb * P:(tb + 1) * P],
                            wv_sb[:, kc, :],
                            start=(kc == 0), stop=(kc == CK - 1),
                        )
                    nc.vector.tensor_copy(v_sb[:, tb, 0:64], ps[:, 0:64])
                    nc.vector.tensor_copy(v_sb[:, tb, 129:257],
                                          ps[:, 64:192])

            # ---- attention + out-projection, interleaved per q-chunk ----
            for qc in range(NQC):
                q0 = qc * QCW
                kbmax = (q0 + QCW - 1) // P
                for h in range(HPC):
                    qp, qci = Q_POS[h]
                    kp, kci = K_POS[h]
                    v0, vw, srow, yrow = V_SLICE[h]
                    psy_t = ps_y_pool.tile([P, QCW], F32, tag="yaug",
                                           name="psy")
                    psy = psy_t[0:vw, :]
                    for kb in range(kbmax + 1):
                        pss = ps_mm.tile([P, QCW], F32, tag="mm")
                        nc.tensor.matmul(
                            pss[:],
                            qkT_sb[kp:kp + D, kci, kb * P:(kb + 1) * P],
                            qkT_sb[qp:qp + D, qci, q0:q0 + QCW],
                            start=True, stop=True,
                        )
                        expT = work.tile([P, QCW], MM_DT, tag="expT")
                        nc.scalar.activation(expT[:], pss[:], Exp,
                                             bias=zb_sb[:])
                        if kb * P >= q0:  # diagonal block
                            nc.vector.tensor_mul(
                                expT[:], expT[:],
                                mask_sb[:, kb - q0 // P, :])
                        nc.tensor.matmul(
                            psy, v_sb[:, kb, v0:v0 + vw], expT[:],
                            start=(kb == 0), stop=(kb == kbmax),
                        )
                    # normalize: yT_h[:, q0:q0+QCW] = y rows / denom
                    den = small.tile([1, QCW], F32, tag="den")
                    nc.scalar.copy(den[:], psy_t[srow:srow + 1, :])
                    recf = small.tile([1, QCW], F32, tag="recf")
                    nc.vector.reciprocal_approx_fast(recf[:], den[:])
                    recip = small.tile([1, QCW], MM_DT, tag="recip")
                    nc.vector.tensor_copy(recip[:], recf[:])
                    psb = ps_bc.tile([P, QCW], F32, tag="bc", name="psb")
                    nc.tensor.matmul(psb[:], ones_sb[:], recip[:],
                                     start=True, stop=True)
                    bc = small.tile([P, QCW], F32, tag="bcs")
                    nc.scalar.copy(bc[yrow:yrow + D, :],
                                   psb[yrow:yrow + D, :])
                    yp, yci = Y_POS[h]
                    nc.vector.tensor_mul(
                        yT_sb[yp:yp + D, yci, q0:q0 + QCW],
                        psy_t[yrow:yrow + D, :], bc[yrow:yrow + D, :])

                # out-projection for the token blocks of this q-chunk
                for tb in range(q0 // P, (q0 + QCW) // P):
                    osb = outp.tile([P, C], F32, tag="osb")
                    for half in range(2):
                        pso = ps_mm.tile([P, 512], F32, tag="mm",
                                         name="pso")[:, :NPH]
                        nc.tensor.matmul(
                            pso, yT_sb[:, 0, tb * P:(tb + 1) * P],
                            wp_sb[:, 0, half * NPH:(half + 1) * NPH],
                            start=True, stop=False)
                        nc.tensor.matmul(
                            pso, yT_sb[0:D, 1, tb * P:(tb + 1) * P],
                            wp_sb[0:D, 1, half * NPH:(half + 1) * NPH],
                            start=False, stop=True)
                        nc.vector.tensor_copy(
                            osb[:, half * NPH:(half + 1) * NPH], pso)
                    nc.sync.dma_start(out[tb * P:(tb + 1) * P, :], osb[:])

    nc.compile()
    return nc


_NC_CACHE = {}


def _get_nc(T=2048, QCW=512):
    key = (T, QCW)
    if key not in _NC_CACHE:
        _NC_CACHE[key] = build_nc(T, QCW)
    return _NC_CACHE[key]


def build_in_maps(inputs):
    """Build the 8 per-core input dicts from full inputs."""
    x = np.asarray(inputs["x"], np.float32)
    W = np.asarray(inputs["W_attn"], np.float32)
    b = np.asarray(inputs["b_attn"], np.float32)
    W_proj = np.asarray(inputs["W_proj"], np.float32)
    in_maps = []
    for c in range(N_CORES):
        bi, g = divmod(c, 4)
        lo = g * (HPC * D)  # local head col offset within each of q/k/v
        qw = [W[:, lo + i * D:lo + (i + 1) * D] * 0.125 for i in range(HPC)]
        kw = [W[:, C + lo + i * D:C + lo + (i + 1) * D] for i in range(HPC)]
        qb = [b[lo + i * D:lo + (i + 1) * D] * 0.125 for i in range(HPC)]
        # chunk order: [q0|q2], [k0|k2], [q1], [k1]
        wqk = np.concatenate([qw[0], qw[2], kw[0], kw[2], qw[1], kw[1]],
                             axis=1)
        z64 = np.zeros(D, np.float32)
        bqk = np.concatenate([qb[0], qb[2], z64, z64, qb[1], z64, z64, z64])
        wv = W[:, 2 * C + lo:2 * C + lo + HPC * D]
        # wp rows: [h0 | h1 | h2 | zero pad] -> chunks (0:128), (128:256)
        wp = np.zeros((2 * P, C), np.float32)
        wp[:HPC * D] = W_proj[lo:lo + HPC * D]
        in_maps.append({
            "xT": np.ascontiguousarray(x[bi].T),
            "wqk": np.ascontiguousarray(wqk),
            "wv": np.ascontiguousarray(wv),
            "bqk": np.ascontiguousarray(bqk),
            "wp": np.ascontiguousarray(wp),
        })
    return in_maps


def postprocess(results, inputs):
    b_attn = np.asarray(inputs["b_attn"], np.float32)
    W_proj = np.asarray(inputs["W_proj"], np.float32)
    b_proj = np.asarray(inputs["b_proj"], np.float32)
    b_eff = b_proj + b_attn[2 * C:] @ W_proj
    T = results[0]["out"].shape[0]
    out = np.zeros((B, T, C), np.float32)
    for c in range(N_CORES):
        out[c // 4] += results[c]["out"]
    out += b_eff
    return out


def kernel(x, W_attn, b_attn, W_proj, b_proj):
    inputs = dict(x=x, W_attn=W_attn, b_attn=b_attn,
                  W_proj=W_proj, b_proj=b_proj)
    T = np.asarray(x).shape[1]
    nc = _get_nc(T=T)
    in_maps = build_in_maps(inputs)
    res = bass_utils.run_bass_kernel_spmd(
        nc, in_maps, core_ids=list(range(N_CORES)))
    return postprocess(res.results, inputs)
